# revision 1
# baseline (speedup 1.0000x reference)
"""AdaIN (segment mean/std + EMA of style stats) on 8 TRN2 NeuronCores — v4.

vs v1 baseline (435us):
  - content is read from HBM ONCE, via gpsimd SWDGE cast-DMAs that land
    f32 HBM data directly into a resident bf16 SBUF cache (no engine cast
    work); style likewise into a resident bf16 cache.  56.7MB/core total.
  - pass-1 segment-sum matmuls 4x column-tiled over the PE array; tiny
    fp32 selector matmul merges the 4 partial stats.
  - pass-2 uses 16-row transposed-one-hot strips: one K=8 broadcast
    matmul covers 32 blocks, one batched is_equal builds 4 gather
    weights, each gather covers 8 blocks (A-half and B-half matmuls into
    separate PSUM banks).  The idx=16 pad sentinel matches no p%16 row,
    so pad rows gather a=b=0 with no dedicated zero rows.
  - FMA: 3 of 4 gathers evac [A|B] to bf16 on ACT then mult+add on DVE
    in 16-bit mode; every 4th reads PSUM directly on DVE (load balance).
  - collectives: dummy AR at t=0 absorbs the first-call barrier; style
    AR overlaps content pass 1; pass-2 prep (index loads + broadcast +
    compare) for the first chunks is emitted before the content-AR
    output DMA so it fills the AR latency.
"""

import os
import sys

import numpy as np

for _p in ("/opt/trn_rl_repo",):
    if _p not in sys.path and os.path.isdir(_p):
        sys.path.insert(0, _p)

from concourse import bacc, bass, bass_utils, masks, mybir, tile

F32 = mybir.dt.float32
BF16 = mybir.dt.bfloat16
I32 = mybir.dt.int32
I16 = mybir.dt.int16

N_CORES = 8
C = 64
B = 16
ALPHA = 0.1
EPS = 1e-8

RC = 128 * 977  # per-core content rows (125056; 8*RC >= 1M)
RS = 128 * 245  # per-core style rows (31360; 8*RS >= 250K)

CH = 32    # pass-1 chunk: blocks per square/onehot tile
SCH = 16   # style pass-1 chunk (f32 HWDGE ring + DVE cast)
LCH = 128  # content cast-DMA load granularity (blocks)
MMB = 2    # pass-1 chunks per matmul burst (keeps PE busy > HAM window)
CH2 = 32   # pass-2 chunk: blocks per out tile (4 gathers of 8)
N_PRE = 8   # pass-2 chunks whose prep is hoisted before the content-AR out
DIRECT_EVERY = 32  # every Nth gather skips the ACT evac (reads PSUM on DVE)

COL_TILE = True


def _ema_lhsT() -> np.ndarray:
    L = np.zeros((B, B), np.float64)
    for b in range(B):
        L[b, 0] = (1.0 - ALPHA) ** b
        for j in range(1, b + 1):
            L[b, j] = ALPHA * (1.0 - ALPHA) ** (b - j)
    return np.ascontiguousarray(L.T).astype(np.float32)


def _e8() -> np.ndarray:
    # e8[g, 16g+s] = 1 : broadcast row g of a K=8 rhs to partitions 16g..16g+16
    e = np.zeros((8, 128), np.float32)
    for g in range(8):
        e[g, 16 * g : 16 * g + 16] = 1.0
    return e


def _s4() -> np.ndarray:
    # s4[32j+s, s] = 1 (s<16): sum the 4 col-group partial stats
    s = np.zeros((128, B), np.float32)
    for j in range(4):
        for t in range(B):
            s[32 * j + t, t] = 1.0
    return s


def _io16() -> np.ndarray:
    return np.broadcast_to(np.arange(B, dtype=np.int16), (128, B)).copy()


def _pid16() -> np.ndarray:
    return (np.arange(128, dtype=np.float32) % 16).reshape(128, 1)


def _chunks(total: int, step: int):
    t0 = 0
    while t0 < total:
        yield t0, min(step, total - t0)
        t0 += step


def build_nc(rc: int = RC, rs: int = RS, n_cores: int = N_CORES):
    ntc = rc // 128
    nts = rs // 128
    ntc_pad = ((ntc + 7) // 8) * 8
    ntc_t = ((ntc + 127) // 128) * 128

    nc = bacc.Bacc(
        "TRN2", target_bir_lowering=False, debug=False, num_devices=n_cores
    )
    cf = nc.dram_tensor("cf", [rc, C], F32, kind="ExternalInput")
    ci = nc.dram_tensor("ci", [rc], I32, kind="ExternalInput")
    sf = nc.dram_tensor("sf", [rs, C], F32, kind="ExternalInput")
    si = nc.dram_tensor("si", [rs], I32, kind="ExternalInput")
    el = nc.dram_tensor("el", [B, B], F32, kind="ExternalInput")
    e8 = nc.dram_tensor("e8", [8, 128], F32, kind="ExternalInput")
    s4 = nc.dram_tensor("s4", [128, B], F32, kind="ExternalInput")
    io16 = nc.dram_tensor("io16", [128, B], I16, kind="ExternalInput")
    pid16 = nc.dram_tensor("pid16", [128, 1], F32, kind="ExternalInput")
    out = nc.dram_tensor("out", [rc, C], BF16, kind="ExternalOutput")

    cf_v = cf.ap().rearrange("(p n) d -> p n d", p=128)
    ci_v = ci.ap().rearrange("(p n) -> p n", p=128)
    sf_v = sf.ap().rearrange("(p n) d -> p n d", p=128)
    si_v = si.ap().rearrange("(p n) -> p n", p=128)
    out_v = out.ap().rearrange("(p n) d -> p n d", p=128)

    def out_q(i):
        return nc.sync if i % 2 == 0 else nc.scalar

    with tile.TileContext(nc) as tc:
        with (
            tc.tile_pool(name="const", bufs=1) as constp,
            tc.tile_pool(name="cache", bufs=1) as cachep,
            tc.tile_pool(name="dram", bufs=1, space="DRAM") as dramp,
        ):
            # ---------- bulk loads, all on the gpsimd SWDGE queue so the
            # descriptor order controls arrival: indices first (everything
            # needs them), then style (its pass 1 + AR run early), then
            # content (largest, consumed as it streams in).  Feature loads
            # are f32->bf16 cast-DMAs into resident caches. ----------
            ci_sb = constp.tile([128, ntc], I32)
            nc.gpsimd.dma_start(ci_sb[:], ci_v)
            si_sb = constp.tile([128, nts], I32)
            nc.gpsimd.dma_start(si_sb[:], si_v)
            sxcache = cachep.tile([128, nts, C], BF16)
            for l0, nl in _chunks(nts, 64):
                nc.gpsimd.dma_start(
                    sxcache[:, l0 : l0 + nl, :], sf_v[:, l0 : l0 + nl, :]
                )
            # content loads are split around the style-AR trigger: the
            # collective_compute instruction rides the same gpsimd queue and
            # waits for style stats, so loads emitted after it would stall —
            # but loads emitted before it keep the SDMA engines fed while
            # the trigger waits.
            xcache = cachep.tile([128, ntc_pad, C], BF16)
            load_list = list(_chunks(ntc, LCH))
            n_half = (len(load_list) + 1) // 2

            def emit_content_loads(lo, hi):
                for l0, nl in load_list[lo:hi]:
                    nc.gpsimd.dma_start(
                        xcache[:, l0 : l0 + nl, :], cf_v[:, l0 : l0 + nl, :]
                    )

            emit_content_loads(0, n_half)

            # ---------- constants ----------
            el_sb = constp.tile([B, B], F32)
            nc.sync.dma_start(el_sb[:], el.ap())
            e8_sbf = constp.tile([8, 128], F32)
            nc.sync.dma_start(e8_sbf[:], e8.ap())
            e8_sb = constp.tile([8, 128], BF16)
            nc.vector.tensor_copy(e8_sb[:], e8_sbf[:])
            s4_sb = constp.tile([128, B], F32)
            nc.sync.dma_start(s4_sb[:], s4.ap())
            io16_sb = constp.tile([128, B], I16)
            nc.sync.dma_start(io16_sb[:], io16.ap())
            pid_sb = constp.tile([128, 1], F32)
            nc.sync.dma_start(pid_sb[:], pid16.ap())
            ident = constp.tile([128, 128], BF16)
            masks.make_identity(nc, ident[:])

            if ntc_pad > ntc:
                nc.vector.memset(xcache[:, ntc:ntc_pad, :], 0.0)

            # ---------- transposed indices -> ciT8 in DRAM:
            # ciT8[g, 128*G+p] = idx of lane p in block 8G+g ----------
            idxbf = constp.tile([128, ntc_t], BF16)
            if ntc_t > ntc:
                nc.vector.memset(idxbf[:, ntc:ntc_t], float(B))
            nc.vector.tensor_copy(idxbf[:, 0:ntc], ci_sb[:])
            ciT = constp.tile([128, ntc_t], BF16)
            ciT8_d = dramp.tile([8, (ntc_t // 8) * 128], BF16, tag="ciT8")
            with tc.tile_pool(name="ps_tr", bufs=2, space="PSUM") as pstr:
                for q in range(ntc_t // 128):
                    psT = pstr.tile([128, 128], BF16, tag="tr")
                    nc.tensor.transpose(
                        psT[:], idxbf[:, 128 * q : 128 * (q + 1)], ident[:]
                    )
                    nc.scalar.copy(ciT[:, 128 * q : 128 * (q + 1)], psT[:])
            # regroup: ciT8[g, (q*16+k)*128 + p] = ciT[8k+g, 128q + p]
            ciT8_view = ciT8_d[:].rearrange("g (q k p) -> g q k p", k=16, p=128)
            for k in range(16):
                nc.sync.dma_start(
                    ciT8_view[:, :, k, :],
                    ciT[8 * k : 8 * k + 8, :].rearrange("g (q p) -> g q p", p=128),
                )

            # ---------- pass-1 helper ----------
            def pass1(x_provider, idx_sb, nt_total, ps, p1w, chunk):
                """Segment sums into ps[32j+s, 0:64]=sum x, [64:128]=sum x^2,
                [128]=count for blocks t%4==j.  One PSUM start/stop per
                col-group (start marks the whole bank pending-zero on the
                written partitions).  Matmuls are emitted in MMB-chunk
                bursts so the PE stays busy past the HAM warm-up window."""
                n_mm = [0, 0, 0, 0]
                mm_tot = [0, 0, 0, 0]
                for t in range(nt_total):
                    mm_tot[t % 4 if COL_TILE else 0] += 2
                n_mm1 = [0, 0, 0, 0]
                mm_tot1 = [0, 0, 0, 0]
                for t in range(nt_total):
                    mm_tot1[t % 4 if COL_TILE else 0] += 1
                pending = []

                def flush():
                    for t0, nb, rhs, oh in pending:
                        for k in range(nb):
                            j = (t0 + k) % 4 if COL_TILE else 0
                            tp = (0, 32 * j) if COL_TILE else None
                            nc.tensor.matmul(
                                ps[32 * j : 32 * j + B, 0 : 2 * C + 1],
                                oh[:, k, :],
                                rhs[:, k, 0 : 2 * C + 1],
                                start=(n_mm1[j] == 0),
                                stop=(n_mm1[j] == mm_tot1[j] - 1),
                                tile_position=tp,
                                skip_group_check=True,
                            )
                            n_mm1[j] += 1
                    pending.clear()

                for ck, (t0, nb) in enumerate(_chunks(nt_total, chunk)):
                    x_ap = x_provider(ck, t0, nb)
                    # rhs = [x | x^2 | 1]: x copied on DVE (bf16 2x), square
                    # on ACT, ones column set once per ring slot
                    rhs = p1w.tile([128, chunk, 132], BF16, tag=f"p1r{chunk}")
                    if ck < MMB + 1:
                        nc.vector.memset(rhs[:, :, 2 * C : 2 * C + 1], 1.0)
                    nc.vector.tensor_copy(rhs[:, :nb, 0:C], x_ap)
                    nc.scalar.activation(
                        rhs[:, :nb, C : 2 * C],
                        x_ap,
                        mybir.ActivationFunctionType.Square,
                    )
                    oh = p1w.tile([128, chunk, B], BF16, tag=f"p1o{chunk}")
                    nc.vector.tensor_tensor(
                        oh[:, :nb, :],
                        idx_sb[:, t0 : t0 + nb]
                        .unsqueeze(2)
                        .broadcast_to((128, nb, B)),
                        io16_sb[:].unsqueeze(1).broadcast_to((128, nb, B)),
                        mybir.AluOpType.is_equal,
                    )
                    pending.append((t0, nb, rhs, oh))
                    if len(pending) >= MMB:
                        flush()
                flush()

            def merge_stats(ps, psel, dst_sb):
                ev = constp.tile([128, 2 * C + 1], F32, tag="ev")
                nc.vector.memset(ev[:], 0.0)
                for j in range(4 if COL_TILE else 1):
                    nc.scalar.copy(
                        ev[32 * j : 32 * j + B, :],
                        ps[32 * j : 32 * j + B, 0 : 2 * C + 1],
                    )
                nc.tensor.matmul(
                    psel[0:B, 0 : 2 * C + 1], s4_sb[:], ev[:], start=True, stop=True
                )
                nc.scalar.copy(dst_sb, psel[0:B, 0 : 2 * C + 1])

            def ar_start(src_sb, tag, w=2 * C + 1):
                inb = dramp.tile([B, w], F32, tag=f"in_{tag}")
                outb = dramp.tile([B, w], F32, tag=f"out_{tag}")
                nc.sync.dma_start(inb[:], src_sb)
                nc.gpsimd.collective_compute(
                    "AllReduce",
                    mybir.AluOpType.add,
                    replica_groups=[list(range(n_cores))],
                    ins=[inb.opt()],
                    outs=[outb.opt()],
                )
                return outb

            def seg_stats(g, mean_out, std_out):
                sums, ssq, cnt = g[:, 0:C], g[:, C : 2 * C], g[:, 2 * C : 2 * C + 1]
                rc_ = constp.tile([B, 1], F32, tag="t1")
                nc.vector.reciprocal(rc_[:], cnt)
                nm1 = constp.tile([B, 1], F32, tag="t2")
                nc.vector.tensor_scalar_add(nm1[:], cnt, -1.0)
                rnm1 = constp.tile([B, 1], F32, tag="t3")
                nc.vector.reciprocal(rnm1[:], nm1[:])
                fac = constp.tile([B, 1], F32, tag="t4")
                nc.vector.tensor_tensor(fac[:], cnt, rnm1[:], mybir.AluOpType.mult)
                nc.vector.tensor_scalar_mul(mean_out, sums, rc_[:])
                ex2 = constp.tile([B, C], F32, tag="t5")
                nc.vector.tensor_scalar_mul(ex2[:], ssq, rc_[:])
                m2 = constp.tile([B, C], F32, tag="t6")
                nc.scalar.square(m2[:], mean_out)
                var = constp.tile([B, C], F32, tag="t7")
                nc.vector.tensor_sub(var[:], ex2[:], m2[:])
                nc.vector.tensor_scalar_mul(var[:], var[:], fac[:])
                nc.vector.tensor_scalar_max(var[:], var[:], 0.0)
                nc.scalar.sqrt(std_out, var[:])
                nc.vector.tensor_scalar_add(std_out, std_out, EPS)

            gstat_s = constp.tile([B, 2 * C + 1], F32)
            gstat_c = constp.tile([B, 2 * C + 1], F32)
            gm_t = constp.tile([B, C], F32)
            gs_t = constp.tile([B, C], F32)

            with (
                tc.tile_pool(name="p1w", bufs=MMB + 1) as p1w,
                tc.tile_pool(name="ps_p1", bufs=1, space="PSUM") as psp,
                tc.tile_pool(name="ps_sel", bufs=1, space="PSUM") as psel_p,
            ):
                # ---------- style pass 1 + early AR ----------
                stat2 = constp.tile([B, 2 * (2 * C + 1)], F32)
                ps_s = psp.tile([128, 512], F32, tag="ps_s")
                pass1(
                    lambda ck, t0, nb: sxcache[:, t0 : t0 + nb, :],
                    si_sb, nts, ps_s, p1w, CH,
                )
                psel = psel_p.tile([128, 2 * C + 1], F32, tag="psel")
                merge_stats(ps_s, psel, stat2[:, 0 : 2 * C + 1])
                emit_content_loads(n_half, None)

                # ---------- content pass 1 ----------
                ps_c = psp.tile([128, 512], F32, tag="ps_c")
                pass1(
                    lambda ck, t0, nb: xcache[:, t0 : t0 + nb, :],
                    ci_sb, ntc, ps_c, p1w, CH,
                )

                psel2 = psel_p.tile([128, 2 * C + 1], F32, tag="psel")
                merge_stats(ps_c, psel2, stat2[:, 2 * C + 1 :])
                outb_c = ar_start(stat2[:], "sc", w=2 * (2 * C + 1))

            # ---------- pass 2 ----------
            chunk_list = list(_chunks(ntc_pad, CH2))

            with (
                tc.tile_pool(name="p2ct", bufs=3) as p2ct,
                tc.tile_pool(name="p2oh", bufs=N_PRE + 2) as p2oh,
                tc.tile_pool(name="p2ab", bufs=3) as p2ab,
                tc.tile_pool(name="p2mt", bufs=4) as p2mt,
                tc.tile_pool(name="p2out", bufs=2) as p2out,
                tc.tile_pool(name="ps_b", bufs=2, space="PSUM") as psb_p,
                tc.tile_pool(name="ps_g", bufs=3, space="PSUM") as psg_p,
            ):
                def p2_prep(t0, nb):
                    """index slice load + K=8 broadcast MM + batched compare
                    -> transposed one-hot strips for nb blocks (nb/8 gathers)"""
                    ngr = nb // 8
                    w = ngr * 128
                    g0 = t0 // 8
                    ct8 = p2ct.tile([8, (CH2 // 8) * 128], BF16, tag="ct8")
                    nc.sync.dma_start(
                        ct8[:, 0:w], ciT8_d[:, g0 * 128 : (g0 + ngr) * 128]
                    )
                    psB = psb_p.tile([128, 512], F32, tag="bc")
                    nc.tensor.matmul(
                        psB[:, 0:w], e8_sb[:], ct8[:, 0:w], start=True, stop=True
                    )
                    ohT = p2oh.tile([128, 512], BF16, tag="ohT")
                    nc.vector.tensor_tensor(
                        ohT[:, 0:w],
                        psB[:, 0:w],
                        pid_sb[:].broadcast_to((128, w)),
                        mybir.AluOpType.is_equal,
                    )
                    return ohT

                # prep for the first chunks runs during the content AR
                preps = {}
                for ck in range(min(N_PRE, len(chunk_list))):
                    t0, nb = chunk_list[ck]
                    preps[ck] = p2_prep(t0, nb)

                # AR out, style EMA, content stats math -> coef
                gstat2 = constp.tile([B, 2 * (2 * C + 1)], F32)
                nc.sync.dma_start(gstat2[:], outb_c[:])
                s_stats = constp.tile([B, 2 * C], F32)
                seg_stats(
                    gstat2[:, 0 : 2 * C + 1], s_stats[:, 0:C], s_stats[:, C : 2 * C]
                )
                g_ps = psb_p.tile([128, 512], F32, tag="bc")
                nc.tensor.matmul(
                    g_ps[0:B, 0 : 2 * C], el_sb[:], s_stats[:], start=True, stop=True
                )
                nc.vector.tensor_copy(gm_t[:], g_ps[0:B, 0:C])
                nc.vector.tensor_copy(gs_t[:], g_ps[0:B, C : 2 * C])
                g_c = gstat2[:, 2 * C + 1 :]
                rc_ = constp.tile([B, 1], F32, tag="t1")
                nc.vector.reciprocal(rc_[:], g_c[:, 2 * C : 2 * C + 1])
                nm1 = constp.tile([B, 1], F32, tag="t2")
                nc.vector.tensor_scalar_add(nm1[:], g_c[:, 2 * C : 2 * C + 1], -1.0)
                rnm1 = constp.tile([B, 1], F32, tag="t3")
                nc.vector.reciprocal(rnm1[:], nm1[:])
                fac = constp.tile([B, 1], F32, tag="t4")
                nc.vector.tensor_tensor(
                    fac[:], g_c[:, 2 * C : 2 * C + 1], rnm1[:], mybir.AluOpType.mult
                )
                mean_c = constp.tile([B, C], F32)
                nc.vector.tensor_scalar_mul(mean_c[:], g_c[:, 0:C], rc_[:])
                ex2 = constp.tile([B, C], F32, tag="t5")
                nc.vector.tensor_scalar_mul(ex2[:], g_c[:, C : 2 * C], rc_[:])
                m2 = constp.tile([B, C], F32, tag="t6")
                nc.scalar.square(m2[:], mean_c[:])
                var = constp.tile([B, C], F32, tag="t7")
                nc.vector.tensor_sub(var[:], ex2[:], m2[:])
                nc.vector.tensor_scalar_mul(var[:], var[:], fac[:])
                std_c = constp.tile([B, C], F32)
                nc.scalar.sqrt(std_c[:], var[:])
                rstd = constp.tile([B, C], F32)
                nc.vector.reciprocal(rstd[:], std_c[:])
                a_t = constp.tile([B, C], F32)
                nc.vector.tensor_tensor(
                    a_t[:], gs_t[:], rstd[:], mybir.AluOpType.mult
                )
                tmp = constp.tile([B, C], F32)
                nc.vector.tensor_tensor(
                    tmp[:], mean_c[:], a_t[:], mybir.AluOpType.mult
                )
                b_t = constp.tile([B, C], F32)
                nc.vector.tensor_sub(b_t[:], gm_t[:], tmp[:])
                coef = constp.tile([B, 2 * C], BF16)
                nc.vector.tensor_copy(coef[:, 0:C], a_t[:])
                nc.vector.tensor_copy(coef[:, C : 2 * C], b_t[:])

                # coef_bd8 [128, 1024]: row 16g+s holds a=coef[s,0:64] at cols
                # [64g,64g+64) in the A half and b=coef[s,64:128] at
                # [512+64g, ...) in the B half; all other entries zero.
                coef_bd = constp.tile([128, 1024], BF16)
                nc.vector.memset(coef_bd[:], 0.0)
                for g in range(8):
                    nc.sync.dma_start(
                        coef_bd[16 * g : 16 * g + B, 64 * g : 64 * g + C],
                        coef[:, 0:C],
                    )
                    nc.sync.dma_start(
                        coef_bd[16 * g : 16 * g + B, 512 + 64 * g : 512 + 64 * g + C],
                        coef[:, C : 2 * C],
                    )

                # ---------- pass-2 main loop ----------
                n_ga = 0
                for ck, (t0, nb) in enumerate(chunk_list):
                    ngr = nb // 8
                    ohT = preps[ck] if ck in preps else p2_prep(t0, nb)
                    ot = p2out.tile([128, CH2, C], BF16, tag="p2o")
                    for u in range(ngr):
                        b0 = t0 + 8 * u
                        psG = psg_p.tile([128, 1024], F32, tag="ga")
                        nc.tensor.matmul(
                            psG[:, 0:512],
                            ohT[:, u * 128 : (u + 1) * 128],
                            coef_bd[:, 0:512],
                            start=True,
                            stop=True,
                        )
                        nc.tensor.matmul(
                            psG[:, 512:1024],
                            ohT[:, u * 128 : (u + 1) * 128],
                            coef_bd[:, 512:1024],
                            start=True,
                            stop=True,
                        )
                        n_ga += 1
                        mt = p2mt.tile([128, 8, C], BF16, tag="mt")
                        path = n_ga % 3
                        if path == 0:
                            # PSUM-direct: DVE mult+add at 1x, no evac
                            av = psG[:, 0:512].rearrange("p (n d) -> p n d", d=C)
                            bv = psG[:, 512:1024].rearrange("p (n d) -> p n d", d=C)
                            add_eng = nc.vector
                        else:
                            # ACT evac to bf16; add on gpsimd for half of
                            # these to spread the elementwise load 3 ways
                            ab = p2ab.tile([128, 1024], BF16, tag="ab")
                            nc.scalar.copy(ab[:], psG[:])
                            av = ab[:, 0:512].rearrange("p (n d) -> p n d", d=C)
                            bv = ab[:, 512:1024].rearrange("p (n d) -> p n d", d=C)
                            add_eng = nc.gpsimd if path == 1 else nc.vector
                        nc.vector.tensor_tensor(
                            mt[:],
                            xcache[:, b0 : b0 + 8, :],
                            av,
                            mybir.AluOpType.mult,
                        )
                        add_eng.tensor_tensor(
                            ot[:, 8 * u : 8 * u + 8, :],
                            mt[:],
                            bv,
                            mybir.AluOpType.add,
                        )
                    nreal = min(nb, ntc - t0)
                    if nreal > 0:
                        out_q(ck).dma_start(
                            out_v[:, t0 : t0 + nreal, :], ot[:, :nreal, :]
                        )

    nc.compile()
    return nc


_NC_CACHE = {}


def _get_nc(rc=RC, rs=RS, n_cores=N_CORES):
    key = (rc, rs, n_cores)
    if key not in _NC_CACHE:
        _NC_CACHE[key] = build_nc(rc, rs, n_cores)
    return _NC_CACHE[key]


def _pad_rows(a: np.ndarray, total: int, fill) -> np.ndarray:
    pad = total - a.shape[0]
    if pad == 0:
        return np.ascontiguousarray(a)
    pad_shape = (pad,) + a.shape[1:]
    return np.concatenate([a, np.full(pad_shape, fill, a.dtype)], axis=0)


def make_in_maps(cf, ci, sf, si, rc=RC, rs=RS, n_cores=N_CORES):
    cf = _pad_rows(np.asarray(cf, np.float32), n_cores * rc, 0.0)
    ci = _pad_rows(np.asarray(ci, np.int32), n_cores * rc, B)
    sf = _pad_rows(np.asarray(sf, np.float32), n_cores * rs, 0.0)
    si = _pad_rows(np.asarray(si, np.int32), n_cores * rs, B)
    el = _ema_lhsT()
    e8 = _e8()
    s4 = _s4()
    io16 = _io16()
    pid16 = _pid16()
    return [
        {
            "cf": np.ascontiguousarray(cf[k * rc : (k + 1) * rc]),
            "ci": np.ascontiguousarray(ci[k * rc : (k + 1) * rc]),
            "sf": np.ascontiguousarray(sf[k * rs : (k + 1) * rs]),
            "si": np.ascontiguousarray(si[k * rs : (k + 1) * rs]),
            "el": el,
            "e8": e8,
            "s4": s4,
            "io16": io16,
            "pid16": pid16,
        }
        for k in range(n_cores)
    ]


def kernel(
    content_feats: np.ndarray,
    style_feats: np.ndarray,
    content_batch_indices: np.ndarray,
    style_batch_indices: np.ndarray,
    num_batches=B,
) -> np.ndarray:
    n_c = content_feats.shape[0]
    nc = _get_nc()
    in_maps = make_in_maps(
        content_feats, content_batch_indices, style_feats, style_batch_indices
    )
    res = bass_utils.run_bass_kernel_spmd(nc, in_maps, core_ids=list(range(N_CORES)))
    out = np.concatenate(
        [np.asarray(res.results[k]["out"]) for k in range(N_CORES)], axis=0
    )
    return np.ascontiguousarray(out[:n_c]).astype(np.float32)

